# revision 3
# baseline (speedup 1.0000x reference)
"""Multi-head graph attention layer (GAT) for Trainium2, 8-core data-parallel.

Problem: B=8, N=1024, D_IN=256, D_OUT=64, H=8, LeakyReLU slope 0.2.
  Wh = einsum('bnd,hdf->bhnf', h, W)
  f1 = Wh @ a1, f2 = Wh @ a2              (per head)
  e  = leaky_relu(f1[:,None] + f2[None,:])
  att = softmax(where(adj==0, -inf, e))
  out = att @ Wh  -> concat heads [B, N, H*F]

Sharding: one batch element per NeuronCore (B=8 across 8 cores).

Core algebra: with x = f1_i + f2_j and exp monotone,
  exp(leaky_relu(x)) = max(exp(x), exp(0.2 x))
                     = E1s_i * E2s_j * max(d_i * c_j, 1)
with d = exp(0.8 f1), c = exp(0.8 f2), E1s = exp(0.2 f1), E2s = exp(0.2 f2).
The E1s_i factor is constant along the softmax axis (j) so it cancels in the
normalization.  The unnormalized weight actually accumulated is

  U[j,i] = m[j,i] * max(E2_j * d_i, E2s_j),   E2 = exp(f2) = E2s * c,

which needs exactly two [N,N]-scale elementwise ops per (head, j-tile):
  t = max(E2_j * dbc, E2s_j)   -- tensor_scalar, per-partition scalars
  U = t * adj                  -- tensor_tensor mask multiply
All [N,N] work is in the transposed U[j,i] layout so U streams directly as
the moving operand of out^T = [Wh | 1]^T @ U; column 64 of out^T is the
softmax denominator Z.  Normalization happens after a PE transpose.

Work is load-balanced across DVE / ACT / GPSIMD via per-(pair, jtile) lanes:
  D: DVE ts (mult,max) + DVE mask TT fused over a head pair
  A: ACT relu(E2*d - E2s) + DVE scalar_tensor_tensor (add E2s, mult mask)
  G: GPSIMD ts + GPSIMD mask TT
"""

import numpy as np
import ml_dtypes

BF16 = ml_dtypes.bfloat16

B, N, D_IN, D_OUT, H = 8, 1024, 256, 64, 8
NEG_SLOPE = 0.2
P = 128                       # partitions
NJT = N // P                  # 8 j-tiles
NIT = N // P                  # 8 i-tiles
NKT = D_IN // P               # 2 k-tiles
HF = H * D_OUT                # 512
AUG = D_OUT + 1               # 65 (Wh columns + ones column)
NPAIR = H // 2                # 4 head pairs

# Engine lane assignment per (pair, jt): 'D' (DVE), 'A' (ACT+DVE),
# 'G' (GPSIMD).  Issue order within a pair follows D, A, G so the PE's
# accumulation group consumes fast lanes first.
LANES = {}
for _p in range(NPAIR):
    for _jt in range(NJT):
        if _jt >= 6:
            LANES[(_p, _jt)] = 'G'
        elif _jt == 5 and _p < 3:
            LANES[(_p, _jt)] = 'A'
        else:
            LANES[(_p, _jt)] = 'D'


def _build_program():
    """Build the single-core SPMD Bass program."""
    import concourse.bass as bass
    import concourse.bacc as bacc
    import concourse.tile as tile
    from concourse import mybir
    from concourse.masks import make_identity

    f32 = mybir.dt.float32
    f32r = mybir.dt.float32r
    bf16 = mybir.dt.bfloat16
    AF = mybir.ActivationFunctionType
    OP = mybir.AluOpType

    nc = bacc.Bacc("TRN2", target_bir_lowering=False, debug=False,
                   enable_asserts=False, num_devices=8)

    hT = nc.dram_tensor("hT", [D_IN, N], f32r, kind="ExternalInput").ap()
    hTb = nc.dram_tensor("hTb", [D_IN, N], bf16, kind="ExternalInput").ap()
    adjT = nc.dram_tensor("adjT", [N, N], bf16, kind="ExternalInput").ap()
    wrsb = nc.dram_tensor("wrsb", [D_IN, HF], bf16,
                          kind="ExternalInput").ap()
    w12 = nc.dram_tensor("w12", [D_IN, 2 * H], f32r,
                         kind="ExternalInput").ap()
    out = nc.dram_tensor("out", [N, HF], f32, kind="ExternalOutput").ap()

    with tile.TileContext(nc) as tc:
        with (
            tc.tile_pool(name="const", bufs=1) as const,
            tc.tile_pool(name="inputs", bufs=1) as inputs,
            tc.tile_pool(name="whp", bufs=1) as whp,
            tc.tile_pool(name="ecol", bufs=1) as ecolp,
            tc.tile_pool(name="ps_ot", bufs=3, space="PSUM") as ps_ot,
            tc.tile_pool(name="ps_tr", bufs=1, space="PSUM") as ps_tr,
            tc.tile_pool(name="bcast", bufs=3) as bcastp,
            tc.tile_pool(name="work", bufs=4) as work,
            tc.tile_pool(name="fin", bufs=3) as fin,
            tc.tile_pool(name="dram", bufs=1, space="DRAM") as dramp,
        ):
            # ---- Phase 0: load inputs -------------------------------------
            ident = const.tile([P, P], f32)
            make_identity(nc, ident)

            ht_sb = []
            for kt in range(NKT):
                t = inputs.tile([P, N], f32r, tag=f"ht{kt}")
                nc.sync.dma_start(out=t, in_=hT[kt * P:(kt + 1) * P, :])
                ht_sb.append(t)
            htb_sb = []
            for kt in range(NKT):
                t = inputs.tile([P, N], bf16, tag=f"htb{kt}")
                nc.sync.dma_start(out=t, in_=hTb[kt * P:(kt + 1) * P, :])
                htb_sb.append(t)
            wrs_sb = []
            for kt in range(NKT):
                t = inputs.tile([P, HF], bf16, tag=f"wrs{kt}")
                nc.sync.dma_start(out=t, in_=wrsb[kt * P:(kt + 1) * P, :])
                wrs_sb.append(t)
            w12_sb = []
            for kt in range(NKT):
                t = inputs.tile([P, 2 * H], f32r, tag=f"w12{kt}")
                nc.sync.dma_start(out=t, in_=w12[kt * P:(kt + 1) * P, :])
                w12_sb.append(t)
            adj_sb = []
            for jt in range(NJT):
                t = inputs.tile([P, N], bf16, tag=f"adj{jt}")
                nc.sync.dma_start(out=t, in_=adjT[jt * P:(jt + 1) * P, :])
                adj_sb.append(t)

            # ---- Phase 1: f scores (transposed), exp scalars --------------
            # fT = w12^T @ hT : rows = [f1 heads | f2 heads], cols = nodes.
            # f1/f2 blocks go to separate PSUM tiles so each sits at base
            # partition 0.
            fT1 = const.tile([H, N], f32)       # f1 rows (for d broadcast)
            fT2 = const.tile([H, N], f32)       # f2 rows (for E2/E2s cols)
            for half in range(2):
                sl = slice(half * 512, (half + 1) * 512)
                pf1 = ps_ot.tile([H, 512], f32, tag="ot")
                pf2 = ps_ot.tile([H, 512], f32, tag="ot")
                for kt in range(NKT):
                    nc.tensor.matmul(pf1, w12_sb[kt][:, 0:H],
                                     ht_sb[kt][:, sl],
                                     start=(kt == 0), stop=(kt == NKT - 1))
                for kt in range(NKT):
                    nc.tensor.matmul(pf2, w12_sb[kt][:, H:2 * H],
                                     ht_sb[kt][:, sl],
                                     start=(kt == 0), stop=(kt == NKT - 1))
                nc.scalar.copy(fT1[:, sl], pf1)
                nc.vector.tensor_copy(fT2[:, sl], pf2)

            # d rows = exp(0.8 f1) -> DRAM for partition broadcasts
            dT = const.tile([H, N], bf16)
            nc.scalar.activation(dT, fT1, AF.Exp, scale=1.0 - NEG_SLOPE)
            dT_dram = dramp.tile([H, N], bf16)
            nc.sync.dma_start(out=dT_dram, in_=dT)

            # per i-tile: transpose f2 chunk, exp -> per-partition scalars
            # ec[:, 0:8] = E2 = exp(f2); ec[:, 8:16] = E2s = exp(0.2 f2);
            # ec[:, 16:24] = -E2s (bias for the ACT relu lane)
            ecols = []
            for it in range(NIT):
                tr8 = ps_tr.tile([P, H], f32, tag="tra")
                nc.tensor.transpose(tr8, fT2[:, it * P:(it + 1) * P],
                                    ident[0:H, 0:H])
                ec = ecolp.tile([P, 3 * H], f32, tag=f"ec{it}")
                nc.scalar.activation(ec[:, 0:H], tr8, AF.Exp, scale=1.0)
                nc.scalar.activation(ec[:, H:2 * H], tr8, AF.Exp,
                                     scale=NEG_SLOPE)
                nc.gpsimd.tensor_scalar(ec[:, 2 * H:3 * H], ec[:, H:2 * H],
                                        -1.0, None, op0=OP.mult)
                ecols.append(ec)

            # ---- Phase 2: Wh = h @ W (bf16), augmented ones column --------
            whaug = []
            for it in range(NIT):
                ps2 = ps_ot.tile([P, H, D_OUT], f32, tag="ot")
                for kt in range(NKT):
                    lhsT = htb_sb[kt][:, it * P:(it + 1) * P]
                    nc.tensor.matmul(ps2, lhsT, wrs_sb[kt],
                                     start=(kt == 0), stop=(kt == NKT - 1))
                wa = whp.tile([P, H, AUG], bf16, tag=f"whaug{it}")
                nc.gpsimd.memset(wa[:, :, D_OUT], 1.0)
                nc.scalar.copy(wa[:, :, 0:D_OUT], ps2)
                whaug.append(wa)

            # output accumulators, one [128, 512] f32 tile per i-tile
            out_sb = []
            for it in range(NIT):
                osb = whp.tile([P, HF], f32, tag=f"osb{it}")
                out_sb.append(osb)

            # ---- Phase 3: per head-pair attention -------------------------
            for p in range(NPAIR):
                h0 = 2 * p
                dbc = bcastp.tile([P, 2, N], bf16, tag="dbc")
                for k in range(2):
                    nc.sync.dma_start(
                        out=dbc[:, k, :],
                        in_=dT_dram[h0 + k:h0 + k + 1, :]
                            .partition_broadcast(P))

                ot = [ps_ot.tile([AUG, N], f32, tag="ot", name=f"ot{k}")
                      for k in range(2)]

                jts = sorted(range(NJT),
                             key=lambda j: {'D': 0, 'A': 1, 'G': 2}[
                                 LANES[(p, j)]])
                first = jts[0]
                last = jts[-1]
                for jt in jts:
                    lane = LANES[(p, jt)]
                    adj = adj_sb[jt]
                    ums = [None, None]
                    if lane == 'D':
                        tp = work.tile([P, 2, N], bf16, tag="tp")
                        for k in range(2):
                            h = h0 + k
                            nc.vector.tensor_scalar(
                                tp[:, k, :], dbc[:, k, :],
                                ecols[jt][:, h:h + 1],
                                ecols[jt][:, H + h:H + h + 1],
                                op0=OP.mult, op1=OP.max)
                        um2 = work.tile([P, 2, N], bf16, tag="um2")
                        adj2 = adj.unsqueeze(1).broadcast_to([P, 2, N])
                        nc.vector.tensor_tensor(out=um2, in0=tp, in1=adj2,
                                                op=OP.mult)
                        ums = [um2[:, 0, :], um2[:, 1, :]]
                    elif lane == 'A':
                        for k in range(2):
                            h = h0 + k
                            r = work.tile([P, N], bf16, tag="ra")
                            nc.scalar.activation(
                                r, dbc[:, k, :], AF.Relu,
                                bias=ecols[jt][:, 2 * H + h:2 * H + h + 1],
                                scale=ecols[jt][:, h:h + 1])
                            um = work.tile([P, N], bf16, tag="uma")
                            nc.vector.scalar_tensor_tensor(
                                um, r, ecols[jt][:, H + h:H + h + 1], adj,
                                op0=OP.add, op1=OP.mult)
                            ums[k] = um
                    else:  # 'G'
                        for k in range(2):
                            h = h0 + k
                            t = work.tile([P, N], bf16, tag="tg")
                            nc.gpsimd.tensor_scalar(
                                t, dbc[:, k, :],
                                ecols[jt][:, h:h + 1],
                                ecols[jt][:, H + h:H + h + 1],
                                op0=OP.mult, op1=OP.max)
                            um = work.tile([P, N], bf16, tag="umg")
                            nc.gpsimd.tensor_tensor(out=um, in0=t, in1=adj,
                                                    op=OP.mult)
                            ums[k] = um
                    for k in range(2):
                        lhsT = whaug[jt][:, h0 + k, :]
                        for nh in range(2):
                            nc.tensor.matmul(
                                ot[k][:, nh * 512:(nh + 1) * 512], lhsT,
                                ums[k][:, nh * 512:(nh + 1) * 512],
                                start=(jt == first), stop=(jt == last))

                # finalize both heads: evacuate, transpose, normalize
                for k in range(2):
                    h = h0 + k
                    ots = fin.tile([AUG, N], f32, tag="ots")
                    nc.scalar.copy(ots, ot[k])
                    tra = ps_tr.tile([P, 4 * AUG], f32, tag="tra")
                    trb = ps_tr.tile([P, 4 * AUG], f32, tag="trb")
                    for it in range(NIT):
                        dst = (tra if it < 4 else trb)[
                            :, (it % 4) * AUG:(it % 4 + 1) * AUG]
                        nc.tensor.transpose(
                            dst, ots[:, it * P:(it + 1) * P],
                            ident[0:AUG, 0:AUG])
                    rc = fin.tile([P, H], f32, tag="rc")
                    nc.vector.reciprocal(
                        rc[:, 0:4], tra[:, D_OUT:4 * AUG:AUG])
                    nc.vector.reciprocal(
                        rc[:, 4:8], trb[:, D_OUT:4 * AUG:AUG])
                    for it in range(NIT):
                        src = (tra if it < 4 else trb)[
                            :, (it % 4) * AUG:(it % 4) * AUG + D_OUT]
                        nc.scalar.activation(
                            out_sb[it][:, h * D_OUT:(h + 1) * D_OUT], src,
                            AF.Copy, scale=rc[:, it:it + 1])

            for it in range(NIT):
                nc.sync.dma_start(out=out[it * P:(it + 1) * P, :],
                                  in_=out_sb[it])

    nc.compile()
    return nc


def _host_prep(h, adj, W, a):
    """Host-side input prep: transposes / casts / tiny einsums only."""
    a1, a2 = a[:, :D_OUT], a[:, D_OUT:]
    w1 = np.einsum("hdf,hf->hd", W, a1).astype(np.float32)   # [H, D_IN]
    w2 = np.einsum("hdf,hf->hd", W, a2).astype(np.float32)
    w12 = np.concatenate([w1.T, w2.T], axis=1)               # [D_IN, 16]
    wrs = np.ascontiguousarray(
        W.transpose(1, 0, 2).reshape(D_IN, HF))
    in_maps = []
    for b in range(B):
        hTf = np.ascontiguousarray(h[b].T).astype(np.float32)
        in_maps.append({
            "hT": hTf,
            "hTb": hTf.astype(BF16),
            "adjT": np.ascontiguousarray(adj[b].T).astype(BF16),
            "wrsb": wrs.astype(BF16),
            "w12": w12,
        })
    return in_maps


def kernel(h, adj, W, a):
    from concourse.bass_utils import run_bass_kernel_spmd

    in_maps = _host_prep(np.asarray(h), np.asarray(adj),
                         np.asarray(W), np.asarray(a))
    nc = _build_program()
    res = run_bass_kernel_spmd(nc, in_maps, core_ids=list(range(B)))
    out = np.stack([np.asarray(res.results[b]["out"]) for b in range(B)])
    return out.astype(np.float32)


# revision 8
# speedup vs baseline: 2.5660x; 2.5660x over previous
"""Multi-head graph attention layer (GAT) for Trainium2, 8-core data-parallel.

Problem: B=8, N=1024, D_IN=256, D_OUT=64, H=8, LeakyReLU slope 0.2.
Sharding: one batch element per NeuronCore.

Core algebra: with x = f1_i + f2_j and exp monotone,
  exp(leaky_relu(x)) = E1s_i * (unnormalized) where the E1s_i factor cancels
in the softmax.  The accumulated unnormalized weight is
  U[j,i] = m[j,i] * max(E2_j * d_i, E2s_j)
with d = exp(0.8 f1), E2 = exp(f2), E2s = exp(0.2 f2).  Two elementwise
passes per (head, j-tile): a (mult,max) tensor_scalar with two per-partition
scalars, then a mask multiply fused over a head pair.  out^T = [Wh|1]^T @ U
gives numerators and the denominator row Z; a PE transpose + batched
broadcast-multiply normalizes.

Per-(pair, jtile) engine lanes:
  D : DVE ts (mult,max) + DVE pair-fused mask TT
  B : ACT relu(E2*d - E2s) + ACT relu(r + E2s) + DVE pair-fused mask TT
  g : DVE ts + GPSIMD mask TT
  G2: whaug E2s-folded; GPSIMD ts (mult ptr, max 1.0) + GPSIMD mask TT
"""

import numpy as np
import ml_dtypes

BF16 = ml_dtypes.bfloat16

B, N, D_IN, D_OUT, H = 8, 1024, 256, 64, 8
NEG_SLOPE = 0.2
P = 128
NJT = N // P
NIT = N // P
NKT = D_IN // P
HF = H * D_OUT                # 512
AUG = D_OUT + 1               # 65
NPAIR = H // 2

LANES = {}
for _p in range(NPAIR):
    for _jt in range(NJT):
        if _jt == 7 and _p == 3:
            LANES[(_p, _jt)] = 'G2'
        elif _jt >= 6:
            LANES[(_p, _jt)] = 'g'
        elif _jt == 5 or (_jt == 4 and _p < 2):
            LANES[(_p, _jt)] = 'B'
        else:
            LANES[(_p, _jt)] = 'D'

# (jt, h) whaug blocks that carry the E2s fold (ones-col becomes E2s), i.e.
# blocks consumed by a G2 lane.
FOLDED = {(jt, 2 * p + k)
          for (p, jt), ln in LANES.items() if ln == 'G2' for k in range(2)}


def _build_program():
    import concourse.bass as bass
    import concourse.bacc as bacc
    import concourse.tile as tile
    from concourse import mybir
    from concourse.masks import make_identity

    f32 = mybir.dt.float32
    f32r = mybir.dt.float32r
    bf16 = mybir.dt.bfloat16
    AF = mybir.ActivationFunctionType
    OP = mybir.AluOpType

    nc = bacc.Bacc("TRN2", target_bir_lowering=False, debug=False,
                   enable_asserts=False, num_devices=8)

    hT = nc.dram_tensor("hT", [D_IN, N], f32r, kind="ExternalInput").ap()
    hTb = nc.dram_tensor("hTb", [D_IN, N], bf16, kind="ExternalInput").ap()
    adjT = nc.dram_tensor("adjT", [N, N], bf16, kind="ExternalInput").ap()
    wrsb = nc.dram_tensor("wrsb", [D_IN, HF], bf16,
                          kind="ExternalInput").ap()
    w12 = nc.dram_tensor("w12", [D_IN, 2 * H], f32r,
                         kind="ExternalInput").ap()
    out = nc.dram_tensor("out", [N, HF], f32, kind="ExternalOutput").ap()

    with tile.TileContext(nc) as tc:
        with (
            tc.tile_pool(name="const", bufs=1) as const,
            tc.tile_pool(name="inputs", bufs=1) as inputs,
            tc.tile_pool(name="whp", bufs=1) as whp,
            tc.tile_pool(name="ecol", bufs=1) as ecolp,
            tc.tile_pool(name="ps_ot", bufs=3, space="PSUM") as ps_ot,
            tc.tile_pool(name="ps_tr", bufs=1, space="PSUM") as ps_tr,
            tc.tile_pool(name="bcast", bufs=3) as bcastp,
            tc.tile_pool(name="work", bufs=4) as work,
            tc.tile_pool(name="fin", bufs=3) as fin,
            tc.tile_pool(name="dram", bufs=1, space="DRAM") as dramp,
        ):
            # ---- Phase 0: loads -------------------------------------------
            ident = const.tile([P, P], f32)
            make_identity(nc, ident)

            ht_sb = []
            for kt in range(NKT):
                t = inputs.tile([P, N], f32r, tag=f"ht{kt}")
                nc.sync.dma_start(out=t, in_=hT[kt * P:(kt + 1) * P, :])
                ht_sb.append(t)
            htb_sb = []
            for kt in range(NKT):
                t = inputs.tile([P, N], bf16, tag=f"htb{kt}")
                nc.sync.dma_start(out=t, in_=hTb[kt * P:(kt + 1) * P, :])
                htb_sb.append(t)
            wrs_sb = []
            for kt in range(NKT):
                t = inputs.tile([P, HF], bf16, tag=f"wrs{kt}")
                nc.sync.dma_start(out=t, in_=wrsb[kt * P:(kt + 1) * P, :])
                wrs_sb.append(t)
            w12_sb = []
            for kt in range(NKT):
                t = inputs.tile([P, 2 * H], f32r, tag=f"w12{kt}")
                nc.sync.dma_start(out=t, in_=w12[kt * P:(kt + 1) * P, :])
                w12_sb.append(t)
            adj_sb = []
            for jt in range(NJT):
                t = inputs.tile([P, N], bf16, tag=f"adj{jt}")
                nc.sync.dma_start(out=t, in_=adjT[jt * P:(jt + 1) * P, :])
                adj_sb.append(t)

            # ---- Phase 1: f scores (transposed), exp scalars --------------
            fT1 = const.tile([H, N], f32)       # f1 rows
            fT2 = const.tile([H, N], f32)       # f2 rows
            for half in range(2):
                sl = slice(half * 512, (half + 1) * 512)
                pf1 = ps_ot.tile([H, 512], f32, tag="ot")
                pf2 = ps_ot.tile([H, 512], f32, tag="ot")
                for kt in range(NKT):
                    nc.tensor.matmul(pf1, w12_sb[kt][:, 0:H],
                                     ht_sb[kt][:, sl],
                                     start=(kt == 0), stop=(kt == NKT - 1))
                for kt in range(NKT):
                    nc.tensor.matmul(pf2, w12_sb[kt][:, H:2 * H],
                                     ht_sb[kt][:, sl],
                                     start=(kt == 0), stop=(kt == NKT - 1))
                nc.scalar.copy(fT1[:, sl], pf1)
                nc.vector.tensor_copy(fT2[:, sl], pf2)

            dT = const.tile([H, N], bf16)
            nc.scalar.activation(dT, fT1, AF.Exp, scale=1.0 - NEG_SLOPE)
            dT_dram = dramp.tile([H, N], bf16)
            nc.sync.dma_start(out=dT_dram, in_=dT)

            # per i-tile scalars: ec[:, 0:8] = E2 = exp(f2),
            # ec[:, 8:16] = E2s = exp(0.2 f2), ec[:, 16:24] = -E2s,
            # ec[:, 24:32] = c = exp(0.8 f2)
            ecols = []
            for it in range(NIT):
                tr8 = ps_tr.tile([P, H], f32, tag="tra")
                nc.tensor.transpose(tr8, fT2[:, it * P:(it + 1) * P],
                                    ident[0:H, 0:H])
                ec = ecolp.tile([P, 4 * H], f32, tag=f"ec{it}")
                nc.scalar.activation(ec[:, 0:H], tr8, AF.Exp, scale=1.0)
                nc.scalar.activation(ec[:, H:2 * H], tr8, AF.Exp,
                                     scale=NEG_SLOPE)
                nc.scalar.activation(ec[:, 3 * H:4 * H], tr8, AF.Exp,
                                     scale=1.0 - NEG_SLOPE)
                nc.gpsimd.tensor_scalar(ec[:, 2 * H:3 * H], ec[:, H:2 * H],
                                        -1.0, None, op0=OP.mult)
                ecols.append(ec)

            # ---- Phase 2: Wh (bf16) with aug column -----------------------
            whaug = []
            for it in range(NIT):
                ps2 = ps_ot.tile([P, H, D_OUT], f32, tag="ot")
                for kt in range(NKT):
                    lhsT = htb_sb[kt][:, it * P:(it + 1) * P]
                    nc.tensor.matmul(ps2, lhsT, wrs_sb[kt],
                                     start=(kt == 0), stop=(kt == NKT - 1))
                wa = whp.tile([P, H, AUG], bf16, tag=f"whaug{it}")
                folded_h = [h for h in range(H) if (it, h) in FOLDED]
                plain_h = [h for h in range(H) if (it, h) not in FOLDED]
                # plain heads: ones column + straight copy (contiguous
                # head ranges only; FOLDED blocks are trailing heads)
                if plain_h:
                    lo, hi = plain_h[0], plain_h[-1] + 1
                    nc.gpsimd.memset(wa[:, lo:hi, D_OUT], 1.0)
                    nc.scalar.copy(wa[:, lo:hi, 0:D_OUT], ps2[:, lo:hi, :])
                for h in folded_h:
                    nc.scalar.activation(
                        wa[:, h, 0:D_OUT], ps2[:, h, :], AF.Copy,
                        scale=ecols[it][:, H + h:H + h + 1])
                    nc.vector.tensor_copy(wa[:, h, D_OUT:AUG],
                                          ecols[it][:, H + h:H + h + 1])
                whaug.append(wa)

            out_big = whp.tile([P, NIT, HF], f32, tag="out_big")

            # ---- Phase 3: per head-pair attention -------------------------
            for p in range(NPAIR):
                h0 = 2 * p
                dbc = bcastp.tile([P, 2, N], bf16, tag="dbc")
                for k in range(2):
                    nc.sync.dma_start(
                        out=dbc[:, k, :],
                        in_=dT_dram[h0 + k:h0 + k + 1, :]
                            .partition_broadcast(P))

                ot = [ps_ot.tile([AUG, N], f32, tag="ot", name=f"ot{k}")
                      for k in range(2)]

                jts = sorted(range(NJT),
                             key=lambda j: {'D': 0, 'B': 1, 'g': 2,
                                            'G2': 3}[LANES[(p, j)]])
                first = jts[0]
                last = jts[-1]
                for jt in jts:
                    lane = LANES[(p, jt)]
                    adj = adj_sb[jt]
                    ums = [None, None]
                    if lane in ('D', 'B'):
                        tp = work.tile([P, 2, N], bf16, tag="tp")
                        for k in range(2):
                            h = h0 + k
                            if lane == 'D':
                                nc.vector.tensor_scalar(
                                    tp[:, k, :], dbc[:, k, :],
                                    ecols[jt][:, h:h + 1],
                                    ecols[jt][:, H + h:H + h + 1],
                                    op0=OP.mult, op1=OP.max)
                            else:
                                r = work.tile([P, N], bf16, tag="rb")
                                nc.scalar.activation(
                                    r, dbc[:, k, :], AF.Relu,
                                    bias=ecols[jt][:,
                                                   2 * H + h:2 * H + h + 1],
                                    scale=ecols[jt][:, h:h + 1])
                                nc.scalar.activation(
                                    tp[:, k, :], r, AF.Relu,
                                    bias=ecols[jt][:, H + h:H + h + 1],
                                    scale=1.0)
                        um2 = work.tile([P, 2, N], bf16, tag="um2")
                        adj2 = adj.unsqueeze(1).broadcast_to([P, 2, N])
                        nc.vector.tensor_tensor(out=um2, in0=tp, in1=adj2,
                                                op=OP.mult)
                        ums = [um2[:, 0, :], um2[:, 1, :]]
                    elif lane == 'g':
                        for k in range(2):
                            h = h0 + k
                            t = work.tile([P, N], bf16, tag="tg")
                            nc.vector.tensor_scalar(
                                t, dbc[:, k, :],
                                ecols[jt][:, h:h + 1],
                                ecols[jt][:, H + h:H + h + 1],
                                op0=OP.mult, op1=OP.max)
                            um = work.tile([P, N], bf16, tag="umg")
                            nc.gpsimd.tensor_tensor(out=um, in0=t, in1=adj,
                                                    op=OP.mult)
                            ums[k] = um
                    else:  # 'G2': fully GPSIMD, whaug E2s-folded
                        for k in range(2):
                            h = h0 + k
                            t = work.tile([P, N], bf16, tag="tg")
                            nc.gpsimd.tensor_scalar(
                                t, dbc[:, k, :],
                                ecols[jt][:, 3 * H + h:3 * H + h + 1],
                                1.0, op0=OP.mult, op1=OP.max)
                            um = work.tile([P, N], bf16, tag="umg")
                            nc.gpsimd.tensor_tensor(out=um, in0=t, in1=adj,
                                                    op=OP.mult)
                            ums[k] = um
                    for k in range(2):
                        lhsT = whaug[jt][:, h0 + k, :]
                        for nh in range(2):
                            nc.tensor.matmul(
                                ot[k][:, nh * 512:(nh + 1) * 512], lhsT,
                                ums[k][:, nh * 512:(nh + 1) * 512],
                                start=(jt == first), stop=(jt == last))

                # finalize: evacuate, transpose, batched normalize
                for k in range(2):
                    h = h0 + k
                    ots = fin.tile([AUG, N], f32, tag="ots")
                    nc.scalar.copy(ots, ot[k])
                    tra = ps_tr.tile([P, 4 * AUG], f32, tag="tra")
                    trb = ps_tr.tile([P, 4 * AUG], f32, tag="trb")
                    for it in range(NIT):
                        dst = (tra if it < 4 else trb)[
                            :, (it % 4) * AUG:(it % 4 + 1) * AUG]
                        nc.tensor.transpose(
                            dst, ots[:, it * P:(it + 1) * P],
                            ident[0:AUG, 0:AUG])
                    rc = fin.tile([P, H], f32, tag="rc")
                    nc.vector.reciprocal(
                        rc[:, 0:4], tra[:, D_OUT:4 * AUG:AUG])
                    nc.vector.reciprocal(
                        rc[:, 4:8], trb[:, D_OUT:4 * AUG:AUG])
                    for g, trx in ((0, tra), (1, trb)):
                        src = trx.rearrange("p (i a) -> p i a",
                                            a=AUG)[:, :, 0:D_OUT]
                        rcb = rc[:, g * 4:(g + 1) * 4].unsqueeze(2) \
                            .broadcast_to([P, 4, D_OUT])
                        dst = out_big[:, g * 4:(g + 1) * 4,
                                      h * D_OUT:(h + 1) * D_OUT]
                        nc.vector.tensor_tensor(out=dst, in0=src, in1=rcb,
                                                op=OP.mult)

            for it in range(NIT):
                nc.sync.dma_start(out=out[it * P:(it + 1) * P, :],
                                  in_=out_big[:, it, :])

    nc.compile()
    return nc


def _host_prep(h, adj, W, a):
    a1, a2 = a[:, :D_OUT], a[:, D_OUT:]
    w1 = np.einsum("hdf,hf->hd", W, a1).astype(np.float32)
    w2 = np.einsum("hdf,hf->hd", W, a2).astype(np.float32)
    w12 = np.concatenate([w1.T, w2.T], axis=1)
    wrs = np.ascontiguousarray(W.transpose(1, 0, 2).reshape(D_IN, HF))
    in_maps = []
    for b in range(B):
        hTf = np.ascontiguousarray(h[b].T).astype(np.float32)
        in_maps.append({
            "hT": hTf,
            "hTb": hTf.astype(BF16),
            "adjT": np.ascontiguousarray(adj[b].T).astype(BF16),
            "wrsb": wrs.astype(BF16),
            "w12": w12,
        })
    return in_maps


def kernel(h, adj, W, a):
    from concourse.bass_utils import run_bass_kernel_spmd

    in_maps = _host_prep(np.asarray(h), np.asarray(adj),
                         np.asarray(W), np.asarray(a))
    nc = _build_program()
    res = run_bass_kernel_spmd(nc, in_maps, core_ids=list(range(B)))
    out = np.stack([np.asarray(res.results[b]["out"]) for b in range(B)])
    return out.astype(np.float32)


# revision 16
# speedup vs baseline: 3.3271x; 1.2966x over previous
"""Multi-head graph attention layer (GAT) for Trainium2, 8-core data-parallel.

Problem: B=8, N=1024, D_IN=256, D_OUT=64, H=8, LeakyReLU slope 0.2.
Sharding: one batch element per NeuronCore.

Core algebra: with x = f1_i + f2_j and exp monotone,
  exp(leaky_relu(x)) = E1s_i * (unnormalized) where the E1s_i factor cancels
in the softmax.  The accumulated unnormalized weight is
  U[j,i] = m[j,i] * max(E2_j * d_i, E2s_j)
with d = exp(0.8 f1), E2 = exp(f2), E2s = exp(0.2 f2).  Two elementwise
passes per (head, j-tile): a (mult,max) tensor_scalar with two per-partition
scalars, then a mask multiply fused over a head pair.  out^T = [Wh|1]^T @ U
gives numerators and the denominator row Z; a PE transpose + batched
broadcast-multiply normalizes.

Per-(pair, jtile) engine lanes:
  D : DVE ts (mult,max) + DVE pair-fused mask TT
  B : ACT relu(E2*d - E2s) + ACT relu(r + E2s) + DVE pair-fused mask TT
  g : DVE ts + GPSIMD mask TT
  G2: whaug E2s-folded; GPSIMD ts (mult ptr, max 1.0) + GPSIMD mask TT
"""

import numpy as np
import ml_dtypes

BF16 = ml_dtypes.bfloat16

B, N, D_IN, D_OUT, H = 8, 1024, 256, 64, 8
NEG_SLOPE = 0.2
P = 128
NJT = N // P
NIT = N // P
NKT = D_IN // P
HF = H * D_OUT                # 512
AUG = D_OUT + 1               # 65
NPAIR = H // 2

LANES = {}
for _p in range(NPAIR):
    for _jt in range(NJT):
        if _jt >= 6:
            LANES[(_p, _jt)] = 'g'
        elif _jt == 5 or (_jt == 4 and _p < 2):
            LANES[(_p, _jt)] = 'B'
        else:
            LANES[(_p, _jt)] = 'D'


def _build_program():
    import concourse.bass as bass
    import concourse.bacc as bacc
    import concourse.tile as tile
    from concourse import mybir
    from concourse.masks import make_identity

    f32 = mybir.dt.float32
    f32r = mybir.dt.float32r
    bf16 = mybir.dt.bfloat16
    AF = mybir.ActivationFunctionType
    OP = mybir.AluOpType

    nc = bacc.Bacc("TRN2", target_bir_lowering=False, debug=False,
                   enable_asserts=False, num_devices=8)

    hT = nc.dram_tensor("hT", [D_IN, N], f32r, kind="ExternalInput").ap()
    hTb = nc.dram_tensor("hTb", [D_IN, N], bf16, kind="ExternalInput").ap()
    adjT = nc.dram_tensor("adjT", [N, N], bf16, kind="ExternalInput").ap()
    wrsb = nc.dram_tensor("wrsb", [D_IN, HF], bf16,
                          kind="ExternalInput").ap()
    w12 = nc.dram_tensor("w12", [D_IN, 2 * H], f32r,
                         kind="ExternalInput").ap()
    out = nc.dram_tensor("out", [N, HF], f32, kind="ExternalOutput").ap()

    with tile.TileContext(nc) as tc:
        with (
            tc.tile_pool(name="const", bufs=1) as const,
            tc.tile_pool(name="inputs", bufs=1) as inputs,
            tc.tile_pool(name="whp", bufs=1) as whp,
            tc.tile_pool(name="ecol", bufs=1) as ecolp,
            tc.tile_pool(name="ps_ot", bufs=3, space="PSUM") as ps_ot,
            tc.tile_pool(name="ps_tr", bufs=1, space="PSUM") as ps_tr,
            tc.tile_pool(name="bcast", bufs=3) as bcastp,
            tc.tile_pool(name="work", bufs=4) as work,
            tc.tile_pool(name="fin", bufs=3) as fin,
            tc.tile_pool(name="dram", bufs=1, space="DRAM") as dramp,
        ):
            # ---- Phase 0: loads -------------------------------------------
            ident = const.tile([P, P], f32)
            make_identity(nc, ident)

            ht_sb = []
            for kt in range(NKT):
                t = inputs.tile([P, N], f32r, tag=f"ht{kt}")
                nc.sync.dma_start(out=t, in_=hT[kt * P:(kt + 1) * P, :])
                ht_sb.append(t)
            htb_sb = []
            for kt in range(NKT):
                t = inputs.tile([P, N], bf16, tag=f"htb{kt}")
                nc.sync.dma_start(out=t, in_=hTb[kt * P:(kt + 1) * P, :])
                htb_sb.append(t)
            wrs_sb = []
            for kt in range(NKT):
                t = inputs.tile([P, HF], bf16, tag=f"wrs{kt}")
                nc.sync.dma_start(out=t, in_=wrsb[kt * P:(kt + 1) * P, :])
                wrs_sb.append(t)
            w12_sb = []
            for kt in range(NKT):
                t = inputs.tile([P, 2 * H], f32r, tag=f"w12{kt}")
                nc.sync.dma_start(out=t, in_=w12[kt * P:(kt + 1) * P, :])
                w12_sb.append(t)
            adj_sb = []
            for jt in range(NJT):
                t = inputs.tile([P, N], bf16, tag=f"adj{jt}")
                nc.sync.dma_start(out=t, in_=adjT[jt * P:(jt + 1) * P, :])
                adj_sb.append(t)

            # ---- Phase 1: f scores (transposed), exp scalars --------------
            fT1 = const.tile([H, N], f32)       # f1 rows
            fT2 = const.tile([H, N], f32)       # f2 rows
            for half in range(2):
                sl = slice(half * 512, (half + 1) * 512)
                pf1 = ps_ot.tile([H, 512], f32, tag="ot")
                pf2 = ps_ot.tile([H, 512], f32, tag="ot")
                for kt in range(NKT):
                    nc.tensor.matmul(pf1, w12_sb[kt][:, 0:H],
                                     ht_sb[kt][:, sl],
                                     start=(kt == 0), stop=(kt == NKT - 1))
                for kt in range(NKT):
                    nc.tensor.matmul(pf2, w12_sb[kt][:, H:2 * H],
                                     ht_sb[kt][:, sl],
                                     start=(kt == 0), stop=(kt == NKT - 1))
                nc.scalar.copy(fT1[:, sl], pf1)
                nc.vector.tensor_copy(fT2[:, sl], pf2)

            dT = const.tile([H, N], bf16)
            nc.scalar.activation(dT, fT1, AF.Exp, scale=1.0 - NEG_SLOPE)
            dT_dram = dramp.tile([H, N], bf16)
            nc.sync.dma_start(out=dT_dram, in_=dT)

            # per i-tile scalars: ec[:, 0:8] = E2 = exp(f2),
            # ec[:, 8:16] = E2s = exp(0.2 f2), ec[:, 16:24] = -E2s,
            # ec[:, 24:32] = c = exp(0.8 f2)
            ecols = []
            for it in range(NIT):
                tr8 = ps_tr.tile([P, H], f32, tag="tra")
                nc.tensor.transpose(tr8, fT2[:, it * P:(it + 1) * P],
                                    ident[0:H, 0:H])
                ec = ecolp.tile([P, 3 * H], f32, tag=f"ec{it}")
                nc.scalar.activation(ec[:, 0:H], tr8, AF.Exp, scale=1.0)
                nc.scalar.activation(ec[:, H:2 * H], tr8, AF.Exp,
                                     scale=NEG_SLOPE)
                nc.gpsimd.tensor_scalar(ec[:, 2 * H:3 * H], ec[:, H:2 * H],
                                        -1.0, None, op0=OP.mult)
                ecols.append(ec)

            # ---- Phase 2: Wh (bf16) with aug column -----------------------
            whaug = []
            for it in range(NIT):
                ps2 = ps_ot.tile([P, H, D_OUT], f32, tag="ot")
                for kt in range(NKT):
                    lhsT = htb_sb[kt][:, it * P:(it + 1) * P]
                    nc.tensor.matmul(ps2, lhsT, wrs_sb[kt],
                                     start=(kt == 0), stop=(kt == NKT - 1))
                wa = whp.tile([P, H, AUG], bf16, tag=f"whaug{it}")
                nc.gpsimd.memset(wa[:, :, D_OUT], 1.0)
                nc.scalar.copy(wa[:, :, 0:D_OUT], ps2)
                whaug.append(wa)

            out_big = whp.tile([P, NIT, HF], f32, tag="out_big")

            # ---- Phase 3: per head-pair attention -------------------------
            # The finalize is software-pipelined: evacuation (ACT) happens
            # right after a pair's last matmul so its PSUM banks free up,
            # but the transposes + normalization are issued one pair later
            # so the engines' queues prioritize the next pair's critical
            # mask/matmul work.
            def _fin_transpose_norm(h0, ots_pair):
                for k in range(2):
                    h = h0 + k
                    ots = ots_pair[k]
                    AUGP = AUG + 1
                    tra = ps_tr.tile([P, 4 * AUGP], f32, tag="tra",
                                     name="tra")
                    trb = ps_tr.tile([P, 4 * AUGP], f32, tag="trb",
                                     name="trb")
                    for it in range(NIT):
                        dst = (tra if it < 4 else trb)[
                            :, (it % 4) * AUGP:(it % 4) * AUGP + AUG]
                        nc.tensor.transpose(
                            dst, ots[:, it * P:(it + 1) * P],
                            ident[0:AUG, 0:AUG])
                    rc = fin.tile([P, H], f32, tag="rc")
                    nc.vector.reciprocal(
                        rc[:, 0:4], tra[:, D_OUT:4 * AUGP:AUGP])
                    nc.vector.reciprocal(
                        rc[:, 4:8], trb[:, D_OUT:4 * AUGP:AUGP])
                    for g, trx in ((0, tra), (1, trb)):
                        src = trx.rearrange("p (i a) -> p i a",
                                            a=AUGP)[:, :, 0:D_OUT]
                        rcb = rc[:, g * 4:(g + 1) * 4].unsqueeze(2) \
                            .broadcast_to([P, 4, D_OUT])
                        dst = out_big[:, g * 4:(g + 1) * 4,
                                      h * D_OUT:(h + 1) * D_OUT]
                        nc.vector.tensor_tensor(out=dst, in0=src, in1=rcb,
                                                op=OP.mult)

            pending = None
            for p in range(NPAIR):
                h0 = 2 * p
                dbc = bcastp.tile([P, 2, N], bf16, tag="dbc")
                for k in range(2):
                    nc.sync.dma_start(
                        out=dbc[:, k, :],
                        in_=dT_dram[h0 + k:h0 + k + 1, :]
                            .partition_broadcast(P))

                ot = [ps_ot.tile([AUG, N], f32, tag="ot", name=f"ot{k}")
                      for k in range(2)]

                jts = sorted(range(NJT),
                             key=lambda j: {'D': 0, 'B': 1, 'g': 2,
                                            'G2': 3}[LANES[(p, j)]])
                first = jts[0]
                last = jts[-1]
                for jt in jts:
                    lane = LANES[(p, jt)]
                    adj = adj_sb[jt]
                    ums = [None, None]
                    if lane in ('D', 'B'):
                        tp = work.tile([P, 2, N], bf16, tag="tp")
                        for k in range(2):
                            h = h0 + k
                            if lane == 'D':
                                nc.vector.tensor_scalar(
                                    tp[:, k, :], dbc[:, k, :],
                                    ecols[jt][:, h:h + 1],
                                    ecols[jt][:, H + h:H + h + 1],
                                    op0=OP.mult, op1=OP.max)
                            else:
                                r = work.tile([P, N], bf16, tag="rb")
                                nc.scalar.activation(
                                    r, dbc[:, k, :], AF.Relu,
                                    bias=ecols[jt][:,
                                                   2 * H + h:2 * H + h + 1],
                                    scale=ecols[jt][:, h:h + 1])
                                nc.scalar.activation(
                                    tp[:, k, :], r, AF.Relu,
                                    bias=ecols[jt][:, H + h:H + h + 1],
                                    scale=1.0)
                        um2 = work.tile([P, 2, N], bf16, tag="um2")
                        adj2 = adj.unsqueeze(1).broadcast_to([P, 2, N])
                        nc.vector.tensor_tensor(out=um2, in0=tp, in1=adj2,
                                                op=OP.mult)
                        ums = [um2[:, 0, :], um2[:, 1, :]]
                    elif lane == 'g':
                        for k in range(2):
                            h = h0 + k
                            t = work.tile([P, N], bf16, tag="tg")
                            nc.vector.tensor_scalar(
                                t, dbc[:, k, :],
                                ecols[jt][:, h:h + 1],
                                ecols[jt][:, H + h:H + h + 1],
                                op0=OP.mult, op1=OP.max)
                            um = work.tile([P, N], bf16, tag="umg")
                            nc.gpsimd.tensor_tensor(out=um, in0=t, in1=adj,
                                                    op=OP.mult)
                            ums[k] = um
                    else:  # 'G2': fully GPSIMD, whaug E2s-folded
                        for k in range(2):
                            h = h0 + k
                            t = work.tile([P, N], bf16, tag="tg")
                            nc.gpsimd.tensor_scalar(
                                t, dbc[:, k, :],
                                ecols[jt][:, 3 * H + h:3 * H + h + 1],
                                1.0, op0=OP.mult, op1=OP.max)
                            um = work.tile([P, N], bf16, tag="umg")
                            nc.gpsimd.tensor_tensor(out=um, in0=t, in1=adj,
                                                    op=OP.mult)
                            ums[k] = um
                    for k in range(2):
                        lhsT = whaug[jt][:, h0 + k, :]
                        for nh in range(2):
                            nc.tensor.matmul(
                                ot[k][:, nh * 512:(nh + 1) * 512], lhsT,
                                ums[k][:, nh * 512:(nh + 1) * 512],
                                start=(jt == first), stop=(jt == last))

                # evacuate this pair's PSUM right away (bf16 numerators)
                ots_pair = []
                for k in range(2):
                    ots = fin.tile([AUG, N], f32, tag="ots", bufs=4,
                                   name=f"ots{k}")
                    nc.scalar.copy(ots, ot[k])
                    ots_pair.append(ots)
                # deferred transpose+normalize of the previous pair
                if pending is not None:
                    _fin_transpose_norm(*pending)
                pending = (h0, ots_pair)

            _fin_transpose_norm(*pending)

            for it in range(NIT):
                nc.sync.dma_start(out=out[it * P:(it + 1) * P, :],
                                  in_=out_big[:, it, :])

    nc.compile()
    return nc


def _host_prep(h, adj, W, a):
    a1, a2 = a[:, :D_OUT], a[:, D_OUT:]
    w1 = np.einsum("hdf,hf->hd", W, a1).astype(np.float32)
    w2 = np.einsum("hdf,hf->hd", W, a2).astype(np.float32)
    w12 = np.concatenate([w1.T, w2.T], axis=1)
    wrs = np.ascontiguousarray(W.transpose(1, 0, 2).reshape(D_IN, HF))
    in_maps = []
    for b in range(B):
        hTf = np.ascontiguousarray(h[b].T).astype(np.float32)
        in_maps.append({
            "hT": hTf,
            "hTb": hTf.astype(BF16),
            "adjT": np.ascontiguousarray(adj[b].T).astype(BF16),
            "wrsb": wrs.astype(BF16),
            "w12": w12,
        })
    return in_maps


def kernel(h, adj, W, a):
    from concourse.bass_utils import run_bass_kernel_spmd

    in_maps = _host_prep(np.asarray(h), np.asarray(adj),
                         np.asarray(W), np.asarray(a))
    nc = _build_program()
    res = run_bass_kernel_spmd(nc, in_maps, core_ids=list(range(B)))
    out = np.stack([np.asarray(res.results[b]["out"]) for b in range(B)])
    return out.astype(np.float32)
